# revision 1
# baseline (speedup 1.0000x reference)
"""Trainium2 Bass kernel for nn_CSCLoss: multi-scale bilinear point-sampling
cosine-consistency loss.

loss = 1 - mean_{pairs,(b,n)} <normalize(sample(feat_i, p_bn)), normalize(sample(feat_j, p_bn))>

Sharding: data-parallel over batch — 32 images -> 8 cores x 4 images; the
host sums the 8 per-core partial sums and applies the 1 - total/count
epilogue (the all-reduce of the sharding hint, done on 8 scalars).

Per-core dataflow (dense, HBM-bandwidth-bound):
 - All per-point scalar math (pixel coords, floor, lerp weights, gather
   indices) runs on partition 0 in [1,128]-wide vector ops from `boxes`.
 - Gather indices are laid out in ap_gather's wrapped format and replicated
   to all 8 DVE 16-partition groups with a 0-stride DRAM->SBUF DMA; bilinear
   weights are replicated to all 128 partitions the same way.
 - Feature maps stream through SBUF as multi-image [128ch, nb*H*W] tiles
   (21 MiB/core at DMA line rate — the roofline) split over two HWDGE rings;
   gpsimd.ap_gather (batched — each dispatch has ~4us fixed cost) extracts
   the 4 bilinear corners per point, DVE applies the lerp weights and
   reduces to sampled vectors v[c, col], col = b*32 + s*4 + rb.
 - Channel reductions (squared norms, pairwise dots) are ones-vector
   matmuls on PE accumulating the two 128-channel chunks into PSUM [1,128].
 - The cosine epilogue runs on partition 0 and emits one [1,1] partial.
"""

import sys
from contextlib import ExitStack

import numpy as np

if "/opt/trn_rl_repo" not in sys.path:
    sys.path.insert(0, "/opt/trn_rl_repo")

B, N, C = 32, 32, 256
LEVELS = [(64, 64), (32, 32), (16, 16)]  # (H, W)
NB = [1, 1, 1]                           # images per gather batch
LORDER = [2, 1, 0]                       # small levels first (early Pool start)
N_CORES = 8
BL = B // N_CORES          # images per core
NPTS = BL * N              # 128 points per core
PAIRS = [(0, 1), (0, 2), (1, 2)]
EPS = 1e-12

_CACHE = {}


def _build_program():
    from concourse import bacc, bass, mybir, tile, library_config

    dt = mybir.dt
    AL = mybir.AluOpType

    nc = bacc.Bacc("TRN2", target_bir_lowering=False, debug=False)

    feats = [
        nc.dram_tensor(f"feat{i}", [BL, C, H, W], dt.float32, kind="ExternalInput")
        for i, (H, W) in enumerate(LEVELS)
    ]
    boxes = nc.dram_tensor("boxes", [BL, N, 4], dt.float32, kind="ExternalInput")
    out = nc.dram_tensor("out", [1, 1], dt.float32, kind="ExternalOutput")

    with tile.TileContext(nc) as tc, ExitStack() as ctx:
        pool = ctx.enter_context(tc.tile_pool(name="sbuf", bufs=1))
        pa = ctx.enter_context(tc.tile_pool(name="pa", bufs=1))
        pstream = ctx.enter_context(tc.tile_pool(name="stream", bufs=1))
        pwork = ctx.enter_context(tc.tile_pool(name="work", bufs=2))
        ppsum = ctx.enter_context(tc.tile_pool(name="psum", bufs=1, space="PSUM"))
        pdram = ctx.enter_context(tc.tile_pool(name="dram", bufs=1, space="DRAM"))

        nc.gpsimd.load_library(library_config.ap_gather)

        # constants for PE-based broadcasts
        ones1 = pool.tile([1, 128], dt.float32)
        nc.vector.memset(ones1[:], 1.0)
        # REPLf[k, q] = 1.0 iff q % 16 == k  (block-replicate [16,*] -> [128,*])
        repl_i = pool.tile([16, 128], dt.int32)
        nc.gpsimd.iota(repl_i[:], pattern=[[1, 128]], base=0, channel_multiplier=15)
        nc.vector.tensor_scalar(
            out=repl_i[:], in0=repl_i[:], scalar1=15, scalar2=None,
            op0=AL.bitwise_and,
        )
        replf = pool.tile([16, 128], dt.float32)
        nc.vector.tensor_scalar(
            out=replf[:], in0=repl_i[:], scalar1=0, scalar2=None, op0=AL.is_equal,
        )

        # ---- boxes load first on the scalar ring (phase A needs it) ----
        bxr = pool.tile([1, BL * N * 4], dt.float32)  # [1, 512] flat boxes
        nc.scalar.dma_start(
            out=bxr[:].rearrange("o (a f) -> o a f", a=BL),
            in_=boxes.rearrange("b n c -> b (n c)"),
        )

        # ---- feature-map streaming DMAs, issued up front ----
        # small levels first on the scalar ring (their gathers start the Pool
        # pipeline early); lvl0 on the sync ring.
        dma_eng = [nc.sync, nc.scalar, nc.scalar]
        T_tiles = {}
        for li in LORDER:
            H, W = LEVELS[li]
            HW = H * W
            nb = NB[li]
            fview = feats[li].rearrange("b c h w -> c b (h w)")
            SBUFS = [5, 8, 8]
            for u in range(BL // nb):
                for ch in range(2):
                    T = pstream.tile(
                        [128, nb * HW], dt.float32, name=f"T{li}_{u}_{ch}",
                        tag=f"T{li}", bufs=SBUFS[li],
                    )
                    dma_eng[li].dma_start(
                        out=T[:].rearrange("c (b q) -> c b q", b=nb),
                        in_=fview[ch * 128:(ch + 1) * 128, u * nb:(u + 1) * nb, :],
                    )
                    T_tiles[(li, u, ch)] = T

        # ---- Phase A: per-point scalar math on partition 0 (DVE) ----
        bxv = bxr[:].rearrange("o (j c) -> o j c", c=4)
        cx = bxv[:, :, 0]  # [1, 128] stride 4
        cy = bxv[:, :, 1]

        def axis_prep(coord, E, ax):
            """pixel coord p=clip(c*(E-1),0,E-1); e0=clamp(floor(p),0,E-2);
            w=p-e0. floor via 16.16 fixed point (exact *2^16; conversion
            error <=2^-16 absorbed by the lerp weight)."""
            pf = pa.tile([1, NPTS], dt.float32, name=f"pf{ax}", tag=f"pf{ax}")
            nc.vector.tensor_scalar(
                out=pf[:], in0=coord, scalar1=float(E - 1), scalar2=0.0,
                op0=AL.mult, op1=AL.max,
            )
            nc.vector.tensor_scalar_min(out=pf[:], in0=pf[:], scalar1=float(E - 1))
            pxs = pa.tile([1, NPTS], dt.float32, name=f"pxs{ax}", tag=f"pxs{ax}")
            nc.vector.tensor_scalar(
                out=pxs[:], in0=pf[:], scalar1=65536.0, scalar2=None, op0=AL.mult,
            )
            ifx = pa.tile([1, NPTS], dt.int32, name=f"ifx{ax}", tag=f"ifx{ax}")
            nc.vector.tensor_copy(out=ifx[:], in_=pxs[:])
            x0i = pa.tile([1, NPTS], dt.int32, name=f"x0i{ax}", tag=f"x0i{ax}")
            nc.vector.tensor_scalar(
                out=x0i[:], in0=ifx[:], scalar1=16, scalar2=None,
                op0=AL.arith_shift_right,
            )
            e0 = pa.tile([1, NPTS], dt.float32, name=f"e0{ax}", tag=f"e0{ax}")
            nc.vector.tensor_copy(out=e0[:], in_=x0i[:])
            nc.vector.tensor_scalar_min(out=e0[:], in0=e0[:], scalar1=float(E - 2))
            we = pa.tile([1, NPTS], dt.float32, name=f"we{ax}", tag=f"we{ax}")
            nc.vector.tensor_tensor(out=we[:], in0=pf[:], in1=e0[:], op=AL.subtract)
            return e0, we

        V = [
            [pool.tile([128, NPTS], dt.float32, name=f"V{li}_{ch}") for ch in range(2)]
            for li in range(3)
        ]
        for li in LORDER:
            H, W = LEVELS[li]
            HW = H * W
            nb = NB[li]
            x0f, wx = axis_prep(cx, W, "x")
            y0f, wy = axis_prep(cy, H, "y")

            # basefu[point(b,n)] = y0*W + x0 + (b % nb)*HW  (unit-local image
            # offset folded in; values < nb*HW <= 16384 fit int16)
            basef = pa.tile([1, NPTS], dt.float32, name="basef", tag="basef")
            nc.vector.tensor_scalar(
                out=basef[:], in0=y0f[:], scalar1=float(W), scalar2=None,
                op0=AL.mult,
            )
            nc.vector.tensor_tensor(
                out=basef[:], in0=basef[:], in1=x0f[:], op=AL.add
            )
            basef_b = basef[:].rearrange("o (b n) -> o b n", b=BL)
            for b in range(BL):
                off = float((b % nb) * HW)
                if off:
                    nc.vector.tensor_scalar(
                        out=basef_b[:, b], in0=basef_b[:, b],
                        scalar1=off, scalar2=None, op0=AL.add,
                    )

            # wrapped index row: flat layout r*32 + b*8 + s, r=rb*4+k,
            # value = basefu[point(b, 4s+rb)] + dk(k), dk = (k//2)*W + k%2
            srow = pa.tile([1, 16 * 32], dt.float32, name="srow", tag="srow")
            srow_v = srow[:].rearrange("o (r b s) -> o r b s", r=16, b=BL)
            basef_v = basef[:].rearrange("o (b s f) -> o b s f", b=BL, f=4)
            for rb in range(4):
                for k in range(4):
                    dk = float((k // 2) * W + (k % 2))
                    nc.vector.tensor_scalar(
                        out=srow_v[:, rb * 4 + k],
                        in0=basef_v[:, :, :, rb],
                        scalar1=dk, scalar2=None, op0=AL.add,
                    )
            sidx = pdram.tile([16, 32], dt.float32, name=f"sidx{li}")
            nc.gpsimd.dma_start(
                out=sidx[:], in_=srow[:].rearrange("o (r c) -> o r c", r=16),
            )
            s16f = pa.tile([16, 32], dt.float32, name="s16f", tag="s16f")
            nc.gpsimd.dma_start(out=s16f[:], in_=sidx[:])
            widx_ps = ppsum.tile([128, 32], dt.float32, name=f"widxps{li}", tag="widxps")
            nc.tensor.matmul(
                widx_ps[:], replf[:], s16f[:], start=True, stop=True,
            )
            widx = pool.tile([128, 32], dt.int16, name=f"widx{li}")
            nc.vector.tensor_copy(out=widx[:], in_=widx_ps[:])

            # corner weights, k = yi*2 + xi, packed k-major then reordered to
            # the gather-output column order (b, s, rb, k)
            w1x = pa.tile([1, NPTS], dt.float32, name="w1x", tag="w1x")
            nc.vector.tensor_scalar(
                out=w1x[:], in0=wx[:], scalar1=-1.0, scalar2=1.0,
                op0=AL.mult, op1=AL.add,
            )
            w1y = pa.tile([1, NPTS], dt.float32, name="w1y", tag="w1y")
            nc.vector.tensor_scalar(
                out=w1y[:], in0=wy[:], scalar1=-1.0, scalar2=1.0,
                op0=AL.mult, op1=AL.add,
            )
            wkt = pa.tile([1, 4 * NPTS], dt.float32, name="wkt", tag="wkt")
            for k, (wyt, wxt) in enumerate(
                [(w1y, w1x), (w1y, wx), (wy, w1x), (wy, wx)]
            ):
                nc.vector.tensor_tensor(
                    out=wkt[:, k * NPTS:(k + 1) * NPTS],
                    in0=wyt[:], in1=wxt[:], op=AL.mult,
                )
            wrow = pa.tile([1, NPTS * 4], dt.float32, name="wrow", tag="wrow")
            # wrow col = b*128 + s*16 + rb*4 + k <- wkt[k*128 + b*32 + s*4 + rb]
            wkt_v = wkt[:].rearrange(
                "o (k b s rb) -> o k b s rb", k=4, b=BL, s=8
            )
            wrow_v = wrow[:].rearrange(
                "o (b s rb k) -> o b s rb k", b=BL, s=8, rb=4
            )
            for b in range(BL):
                nc.vector.tensor_copy(
                    out=wrow_v[:, b],
                    in_=wkt_v[:, :, b].rearrange("o k s rb -> o s rb k"),
                )
            wb_ps = ppsum.tile([128, NPTS * 4], dt.float32, name=f"wbps{li}", tag="wbps")
            nc.tensor.matmul(wb_ps[:], ones1[:], wrow[:], start=True, stop=True)
            wb = pool.tile([128, NPTS * 4], dt.float32, name=f"wb{li}")
            nc.vector.tensor_copy(out=wb[:], in_=wb_ps[:])
            # ---- this level's gathers + lerp (V col = b*32 + s*4 + rb) ----
            ncols = nb * 128
            for u in range(BL // nb):
                for ch in range(2):
                    T = T_tiles[(li, u, ch)]
                    og = pwork.tile(
                        [128, ncols], dt.float32, name=f"og{li}", tag="og"
                    )
                    nc.gpsimd.ap_gather(
                        out_ap=og[:], in_ap=T[:],
                        idxs_ap=widx[:, u * nb * 8:(u + 1) * nb * 8],
                        channels=128, num_elems=nb * HW, d=1, num_idxs=ncols,
                    )
                    nc.vector.tensor_tensor(
                        out=og[:], in0=og[:],
                        in1=wb[:, u * ncols:(u + 1) * ncols], op=AL.mult,
                    )
                    nc.vector.tensor_reduce(
                        out=V[li][ch][:, u * nb * 32:(u + 1) * nb * 32],
                        in_=og[:].rearrange("c (n f) -> c n f", f=4),
                        axis=mybir.AxisListType.X, op=AL.add,
                    )

        # ---- Phase C: channel reductions via ones-matmul into PSUM ----
        ones = pool.tile([128, 1], dt.float32)
        nc.vector.memset(ones[:], 1.0)

        def colsum(name, make_in):
            ps = ppsum.tile([1, NPTS], dt.float32, name=name)
            for ch in range(2):
                prod = pwork.tile(
                    [128, NPTS], dt.float32, name=f"prod{name}{ch}", tag="prod"
                )
                make_in(prod, ch)
                nc.tensor.matmul(
                    ps[:], ones[:], prod[:], start=(ch == 0), stop=(ch == 1),
                )
            sb = pool.tile([1, NPTS], dt.float32, name=f"sb{name}")
            nc.vector.tensor_copy(out=sb[:], in_=ps[:])
            return sb

        ss = [
            colsum(
                f"ss{li}",
                lambda prod, ch, li=li: nc.vector.tensor_tensor(
                    out=prod[:], in0=V[li][ch][:], in1=V[li][ch][:], op=AL.mult
                ),
            )
            for li in range(3)
        ]
        dots = {}
        for i, j in PAIRS:
            dots[(i, j)] = colsum(
                f"d{i}{j}",
                lambda prod, ch, i=i, j=j: nc.vector.tensor_tensor(
                    out=prod[:], in0=V[i][ch][:], in1=V[j][ch][:], op=AL.mult
                ),
            )

        # ---- Phase D: cosine epilogue on partition 0 ----
        rns = []
        for li in range(3):
            nrm = pool.tile([1, NPTS], dt.float32, name=f"nrm{li}")
            nc.scalar.sqrt(out=nrm[:], in_=ss[li][:])
            nc.vector.tensor_scalar_max(out=nrm[:], in0=nrm[:], scalar1=EPS)
            rn = pool.tile([1, NPTS], dt.float32, name=f"rn{li}")
            nc.vector.reciprocal(out=rn[:], in_=nrm[:])
            rns.append(rn)

        tot = pool.tile([1, NPTS], dt.float32)
        first = True
        for i, j in PAIRS:
            t = pool.tile([1, NPTS], dt.float32, name=f"t{i}{j}")
            nc.vector.tensor_tensor(
                out=t[:], in0=dots[(i, j)][:], in1=rns[i][:], op=AL.mult
            )
            nc.vector.tensor_tensor(out=t[:], in0=t[:], in1=rns[j][:], op=AL.mult)
            if first:
                nc.vector.tensor_copy(out=tot[:], in_=t[:])
                first = False
            else:
                nc.vector.tensor_tensor(out=tot[:], in0=tot[:], in1=t[:], op=AL.add)

        res = pool.tile([1, 1], dt.float32)
        nc.vector.tensor_reduce(
            out=res[:], in_=tot[:], axis=mybir.AxisListType.X, op=AL.add
        )
        nc.sync.dma_start(out=out.ap(), in_=res[:])

    nc.compile()
    return nc


def _get_program():
    if "nc" not in _CACHE:
        _CACHE["nc"] = _build_program()
    return _CACHE["nc"]


def _run_device(feat0, feat1, feat2, boxes, **run_kwargs):
    """Shard inputs batch-wise over the 8 cores, run the SPMD program, and
    return the BassKernelResults (one {"out": [1,1]} per core)."""
    from concourse.bass_utils import run_bass_kernel_spmd

    nc = _get_program()

    feats = [
        np.ascontiguousarray(np.asarray(f, dtype=np.float32))
        for f in (feat0, feat1, feat2)
    ]
    boxes = np.ascontiguousarray(np.asarray(boxes, dtype=np.float32))

    in_maps = []
    for k in range(N_CORES):
        sl = slice(k * BL, (k + 1) * BL)
        in_maps.append(
            {
                "feat0": feats[0][sl],
                "feat1": feats[1][sl],
                "feat2": feats[2][sl],
                "boxes": boxes[sl],
            }
        )

    return run_bass_kernel_spmd(
        nc, in_maps, core_ids=list(range(N_CORES)), **run_kwargs
    )


def kernel(feat0, feat1, feat2, boxes):
    r = _run_device(feat0, feat1, feat2, boxes)
    total = np.float64(0.0)
    for m in r.results:
        total += np.float64(m["out"].reshape(-1)[0])

    count = B * N * len(PAIRS)
    avg = np.float32(total) / np.float32(count)
    loss = np.float32(1.0) - avg
    loss = np.nan_to_num(loss, nan=0.0, posinf=1.0, neginf=0.0)
    return np.array(np.clip(loss, 0.0, 2.0), dtype=np.float32)



# revision 6
# speedup vs baseline: 1.0738x; 1.0738x over previous
"""Trainium2 Bass kernel for nn_CSCLoss: multi-scale bilinear point-sampling
cosine-consistency loss.

loss = 1 - mean_{pairs,(b,n)} <normalize(sample(feat_i, p_bn)), normalize(sample(feat_j, p_bn))>

Sharding: data-parallel over batch — 32 images -> 8 cores x 4 images; the
host sums the 8 per-core partial sums and applies the 1 - total/count
epilogue (the all-reduce of the sharding hint, done on 8 scalars).

Per-core dataflow (DMA- and gather-balanced, ~ridge):
 - Feature maps stream through SBUF as merged-channel-half tiles
   [128ch, 2*H*W]; L0 (64x64, 16 MiB) on one HWDGE ring, L1+L2 on the other.
 - gpsimd.ap_gather extracts bilinear corners. Its cost is per-INDEX
   (~28 ns/idx), so the index count is the knob:
    * L0: plain f32 gather, 4 corner idxs per point per ch-half, one
      256-idx dispatch per image (runs under L0's own DMA time).
    * L1/L2: tiles are re-packed into bf16 PAIR arrays (u32 word p holds
      bf16 pixels (p, p+1); even pairs are the bf16 cast itself, odd pairs
      are built with two strided u16 copies on the otherwise-idle ACT
      engine). One u32 gather then yields BOTH x-corners -> 2 idxs per
      point per ch-half, one 512-idx dispatch per level.
 - Gather order L2 -> L1 -> L0-images keeps gpsimd busy from ~8 us while
   L0 is still streaming in.
 - DVE applies lerp weights (PE-broadcast to 128 partitions) and reduces
   corner groups of 4 to sampled vectors V[c, u*64 + half*32 + n].
 - Channel reductions (squared norms, pairwise dots) are ones-vector
   matmuls accumulating [1, 256] in PSUM, + a strided add folding the two
   channel halves; the cosine epilogue runs on partition 0 and emits one
   [1,1] partial per core.
"""

import sys
from contextlib import ExitStack

import numpy as np

if "/opt/trn_rl_repo" not in sys.path:
    sys.path.insert(0, "/opt/trn_rl_repo")

B, N, C = 32, 32, 256
LEVELS = [(64, 64), (32, 32), (16, 16)]  # (H, W)
N_CORES = 8
BL = B // N_CORES          # images per core
NPTS = BL * N              # 128 points per core
PAIRS = [(0, 1), (0, 2), (1, 2)]
EPS = 1e-12

_CACHE = {}


def _build_program():
    from concourse import bacc, bass, mybir, tile, library_config

    dt = mybir.dt
    AL = mybir.AluOpType

    nc = bacc.Bacc("TRN2", target_bir_lowering=False, debug=False)

    feats = [
        nc.dram_tensor(f"feat{i}", [BL, C, H, W], dt.float32, kind="ExternalInput")
        for i, (H, W) in enumerate(LEVELS)
    ]
    boxes = nc.dram_tensor("boxes", [BL, N, 4], dt.float32, kind="ExternalInput")
    out = nc.dram_tensor("out", [1, 1], dt.float32, kind="ExternalOutput")

    with tile.TileContext(nc) as tc, ExitStack() as ctx:
        pool = ctx.enter_context(tc.tile_pool(name="sbuf", bufs=1))
        pa = ctx.enter_context(tc.tile_pool(name="pa", bufs=1))
        pstream = ctx.enter_context(tc.tile_pool(name="stream", bufs=1))
        pwork = ctx.enter_context(tc.tile_pool(name="work", bufs=2))
        ppsum = ctx.enter_context(tc.tile_pool(name="psum", bufs=1, space="PSUM"))
        pdram = ctx.enter_context(tc.tile_pool(name="dram", bufs=1, space="DRAM"))

        nc.gpsimd.load_library(library_config.ap_gather)

        # ---- constants ----
        ones1 = pool.tile([1, 128], dt.float32)
        nc.vector.memset(ones1[:], 1.0)
        # REPLf[k, q] = 1.0 iff q % 16 == k  (block-replicate [16,*] -> [128,*])
        repl_i = pool.tile([16, 128], dt.int32)
        nc.gpsimd.iota(repl_i[:], pattern=[[1, 128]], base=0, channel_multiplier=15)
        nc.vector.tensor_scalar(
            out=repl_i[:], in0=repl_i[:], scalar1=15, scalar2=None,
            op0=AL.bitwise_and,
        )
        replf = pool.tile([16, 128], dt.float32)
        nc.vector.tensor_scalar(
            out=replf[:], in0=repl_i[:], scalar1=0, scalar2=None, op0=AL.is_equal,
        )
        ones = pool.tile([128, 1], dt.float32)
        nc.vector.memset(ones[:], 1.0)

        # ---- boxes first on the sync ring ----
        bxr = pool.tile([1, NPTS * 4], dt.float32)
        nc.sync.dma_start(
            out=bxr[:].rearrange("o (a f) -> o a f", a=BL),
            in_=boxes.rearrange("b n c -> b (n c)"),
        )

        # ---- feature streaming DMAs ----
        # merged-half views: [c=128, b, (h, hw)]
        fviews = [
            feats[li].rearrange("b (h c) hh ww -> c b h (hh ww)", h=2)
            for li in range(3)
        ]
        # sync ring: L2 imgs then L1 imgs (small levels first -> early gathers)
        T_tiles = {}
        for li, bufs in ((2, 2), (1, 2)):
            H, W = LEVELS[li]
            HW = H * W
            for u in range(BL):
                T = pstream.tile(
                    [128, 2 * HW], dt.float32, name=f"T{li}_{u}",
                    tag=f"T{li}", bufs=bufs,
                )
                nc.sync.dma_start(
                    out=T[:].rearrange("c (h q) -> c h q", h=2),
                    in_=fviews[li][:, u],
                )
                T_tiles[(li, u)] = T
        # scalar ring: all of L0
        H0, W0 = LEVELS[0]
        HW0 = H0 * W0
        for u in range(BL):
            T = pstream.tile(
                [128, 2 * HW0], dt.float32, name=f"T0_{u}", tag="T0", bufs=2,
            )
            nc.scalar.dma_start(
                out=T[:].rearrange("c (h q) -> c h q", h=2),
                in_=fviews[0][:, u],
            )
            T_tiles[(0, u)] = T

        # ---- phase A helpers (partition-0 [1,*] math) ----
        bxv = bxr[:].rearrange("o (j c) -> o j c", c=4)
        cx = bxv[:, :, 0]  # [1, 128] stride 4, pt = u*32+n
        cy = bxv[:, :, 1]

        def axis_prep(coord, E, ax):
            """p = clip(c*(E-1), 0, E-1); e0 = clamp(floor(p), 0, E-2); w = p-e0.
            floor via 16.16 fixed point."""
            pf = pa.tile([1, NPTS], dt.float32, name=f"pf{ax}", tag=f"pf{ax}")
            nc.vector.tensor_scalar(
                out=pf[:], in0=coord, scalar1=float(E - 1), scalar2=0.0,
                op0=AL.mult, op1=AL.max,
            )
            nc.vector.tensor_scalar_min(out=pf[:], in0=pf[:], scalar1=float(E - 1))
            pxs = pa.tile([1, NPTS], dt.float32, name=f"pxs{ax}", tag=f"pxs{ax}")
            nc.vector.tensor_scalar(
                out=pxs[:], in0=pf[:], scalar1=65536.0, scalar2=None, op0=AL.mult,
            )
            ifx = pa.tile([1, NPTS], dt.int32, name=f"ifx{ax}", tag=f"ifx{ax}")
            nc.vector.tensor_copy(out=ifx[:], in_=pxs[:])
            x0i = pa.tile([1, NPTS], dt.int32, name=f"x0i{ax}", tag=f"x0i{ax}")
            nc.vector.tensor_scalar(
                out=x0i[:], in0=ifx[:], scalar1=16, scalar2=None,
                op0=AL.arith_shift_right,
            )
            e0 = pa.tile([1, NPTS], dt.float32, name=f"e0{ax}", tag=f"e0{ax}")
            nc.vector.tensor_copy(out=e0[:], in_=x0i[:])
            nc.vector.tensor_scalar_min(out=e0[:], in0=e0[:], scalar1=float(E - 2))
            we = pa.tile([1, NPTS], dt.float32, name=f"we{ax}", tag=f"we{ax}")
            nc.vector.tensor_tensor(out=we[:], in0=pf[:], in1=e0[:], op=AL.subtract)
            return e0, we

        def corner_weights(wx, wy, name):
            """wkt [1, 4*NPTS]: (k, pt) with k = yi*2 + xi."""
            w1x = pa.tile([1, NPTS], dt.float32, name=f"w1x{name}", tag="w1x")
            nc.vector.tensor_scalar(
                out=w1x[:], in0=wx[:], scalar1=-1.0, scalar2=1.0,
                op0=AL.mult, op1=AL.add,
            )
            w1y = pa.tile([1, NPTS], dt.float32, name=f"w1y{name}", tag="w1y")
            nc.vector.tensor_scalar(
                out=w1y[:], in0=wy[:], scalar1=-1.0, scalar2=1.0,
                op0=AL.mult, op1=AL.add,
            )
            wkt = pa.tile([1, 4 * NPTS], dt.float32, name=f"wkt{name}", tag="wkt")
            for k, (wyt, wxt) in enumerate(
                [(w1y, w1x), (w1y, wx), (wy, w1x), (wy, wx)]
            ):
                nc.vector.tensor_tensor(
                    out=wkt[:, k * NPTS:(k + 1) * NPTS],
                    in0=wyt[:], in1=wxt[:], op=AL.mult,
                )
            return wkt

        def wrap_idx(srow, n_idx, name):
            """srow [1, n_idx] (pos p = (j%16)*(n_idx//16) + j//16) ->
            widx [128, n_idx//16] int16 replicated to all 8 cores."""
            X = n_idx // 16
            sidx = pdram.tile([16, X], dt.float32, name=f"sidx{name}")
            nc.gpsimd.dma_start(
                out=sidx[:], in_=srow[:].rearrange("o (r c) -> o r c", r=16),
            )
            s16f = pa.tile([16, X], dt.float32, name=f"s16f{name}", tag=f"s16f{name}")
            nc.gpsimd.dma_start(out=s16f[:], in_=sidx[:])
            widx_ps = ppsum.tile(
                [128, X], dt.float32, name=f"widxps{name}", tag="widxps", bufs=2,
            )
            nc.tensor.matmul(widx_ps[:], replf[:], s16f[:], start=True, stop=True)
            widx = pool.tile([128, X], dt.int16, name=f"widx{name}")
            nc.vector.tensor_copy(out=widx[:], in_=widx_ps[:])
            return widx

        def broadcast_weights(wrow, n_col, name, out_dt):
            """wrow [1, n_col] -> wb [128, n_col] via PE rank-1 broadcast."""
            wb = pool.tile([128, n_col], out_dt, name=f"wb{name}")
            for c0 in range(0, n_col, 512):
                cw = min(512, n_col - c0)
                wb_ps = ppsum.tile(
                    [128, cw], dt.float32, name=f"wbps{name}_{c0}", tag="wbps",
                    bufs=2,
                )
                nc.tensor.matmul(
                    wb_ps[:], ones1[:], wrow[:, c0:c0 + cw], start=True, stop=True,
                )
                nc.vector.tensor_copy(out=wb[:, c0:c0 + cw], in_=wb_ps[:])
            return wb

        # V tiles: col = u*64 + h*32 + n
        V = [pool.tile([128, 256], dt.float32, name=f"V{li}") for li in range(3)]

        # =========== phase A / packing / index prep, small levels first =====
        packed = {}   # li -> (P32 tile, widx, wb)
        for li in (2, 1):
            H, W = LEVELS[li]
            HW = H * W
            HW2 = HW // 2
            x0f, wx = axis_prep(cx, W, f"x{li}")
            y0f, wy = axis_prep(cy, H, f"y{li}")
            # q = y0*W + x0 (integer-valued f32)
            qf = pa.tile([1, NPTS], dt.float32, name=f"qf{li}", tag="qf")
            nc.vector.tensor_scalar(
                out=qf[:], in0=y0f[:], scalar1=float(W), scalar2=None, op0=AL.mult,
            )
            nc.vector.tensor_tensor(out=qf[:], in0=qf[:], in1=x0f[:], op=AL.add)
            qi = pa.tile([1, NPTS], dt.int32, name=f"qi{li}", tag="qi")
            nc.vector.tensor_copy(out=qi[:], in_=qf[:])
            pari = pa.tile([1, NPTS], dt.int32, name=f"pari{li}", tag="pari")
            nc.vector.tensor_scalar(
                out=pari[:], in0=qi[:], scalar1=1, scalar2=None, op0=AL.bitwise_and,
            )
            shi = pa.tile([1, NPTS], dt.int32, name=f"shi{li}", tag="shi")
            nc.vector.tensor_scalar(
                out=shi[:], in0=qi[:], scalar1=1, scalar2=None,
                op0=AL.arith_shift_right,
            )
            parf = pa.tile([1, NPTS], dt.float32, name=f"parf{li}", tag="parf")
            nc.vector.tensor_copy(out=parf[:], in_=pari[:])
            shf = pa.tile([1, NPTS], dt.float32, name=f"shf{li}", tag="shf")
            nc.vector.tensor_copy(out=shf[:], in_=shi[:])
            # slot = (q>>1) + (q&1)*HW2
            slotf = pa.tile([1, NPTS], dt.float32, name=f"slotf{li}", tag="slotf")
            nc.vector.tensor_scalar(
                out=slotf[:], in0=parf[:], scalar1=float(HW2), scalar2=None,
                op0=AL.mult,
            )
            nc.vector.tensor_tensor(out=slotf[:], in0=slotf[:], in1=shf[:], op=AL.add)
            # slotseg [1, 256] (h, pt): slot + (u*2+h)*HW
            slotseg = pa.tile([1, 2 * NPTS], dt.float32, name=f"sseg{li}", tag="sseg")
            for h in range(2):
                for u in range(BL):
                    nc.vector.tensor_scalar(
                        out=slotseg[:, h * NPTS + u * N: h * NPTS + (u + 1) * N],
                        in0=slotf[:, u * N:(u + 1) * N],
                        scalar1=float((u * 2 + h) * HW), scalar2=None, op0=AL.add,
                    )
            # srow [1, 512]: p = (j%16)*32 + j//16, j = u*128 + h*64 + n*2 + row
            srow = pa.tile([1, 512], dt.float32, name=f"srow{li}", tag="srow")
            srow_v = srow[:].rearrange(
                "o (nl row u h nh) -> o nl row u h nh", nl=8, row=2, u=BL, h=2,
            )
            for row in range(2):
                for h in range(2):
                    nc.vector.tensor_scalar(
                        out=srow_v[:, :, row, :, h, :],
                        in0=slotseg[:, h * NPTS:(h + 1) * NPTS].rearrange(
                            "o (u nh nl) -> o nl u nh", u=BL, nh=4,
                        ),
                        scalar1=float(row * (W // 2)), scalar2=None, op0=AL.add,
                    )
            widx = wrap_idx(srow, 512, f"L{li}")
            # weights wrow [1, 1024]: col (u, h, n, row, xi) = wkt[row*2+xi, pt]
            wkt = corner_weights(wx, wy, f"L{li}")
            wrow = pa.tile([1, 1024], dt.float32, name=f"wrow{li}", tag="wrow")
            wrow_v = wrow[:].rearrange(
                "o (u h n row xi) -> o u h n row xi", u=BL, h=2, n=N, row=2,
            )
            for h in range(2):
                for row in range(2):
                    for xi in range(2):
                        k = row * 2 + xi
                        nc.vector.tensor_copy(
                            out=wrow_v[:, :, h, :, row, xi],
                            in_=wkt[:, k * NPTS:(k + 1) * NPTS].rearrange(
                                "o (u n) -> o u n", u=BL,
                            ),
                        )
            wb = broadcast_weights(wrow, 1024, f"L{li}", dt.bfloat16)

            # packing: P32 [128, 8*HW] u32; seg = u*2+h
            P32 = pool.tile([128, 8 * HW], dt.int32, name=f"P32_{li}")
            Pb = P32[:].bitcast(dt.bfloat16)  # [128, 16*HW]
            for u in range(BL):
                T = T_tiles[(li, u)]
                for h in range(2):
                    seg = u * 2 + h
                    base = seg * 2 * HW
                    # even pairs: the bf16 cast itself
                    nc.vector.tensor_copy(
                        out=Pb[:, base:base + HW],
                        in_=T[:, h * HW:(h + 1) * HW],
                    )
                    # odd pairs on ACT: P_odd16[2k] = B16[2k+1] (k < HW2),
                    #                   P_odd16[2k+1] = B16[2k+2] (k < HW2-1)
                    bview = Pb[:, base:base + HW].rearrange(
                        "c (p two) -> c p two", two=2,
                    )
                    oview = Pb[:, base + HW:base + 2 * HW].rearrange(
                        "c (p two) -> c p two", two=2,
                    )
                    nc.scalar.copy(out=oview[:, :, 0], in_=bview[:, :, 1])
                    nc.scalar.copy(
                        out=oview[:, 0:HW2 - 1, 1], in_=bview[:, 1:HW2, 0],
                    )
            packed[li] = (P32, widx, wb)

        # =========== L0 phase A ===========
        x0f, wx = axis_prep(cx, W0, "x0")
        y0f, wy = axis_prep(cy, H0, "y0")
        qf0 = pa.tile([1, NPTS], dt.float32, name="qf0", tag="qf")
        nc.vector.tensor_scalar(
            out=qf0[:], in0=y0f[:], scalar1=float(W0), scalar2=None, op0=AL.mult,
        )
        nc.vector.tensor_tensor(out=qf0[:], in0=qf0[:], in1=x0f[:], op=AL.add)
        # srow0 [1, 1024]: p = u*256 + (j%16)*16 + j//16, j = h*128 + n*4 + k
        srow0 = pa.tile([1, 1024], dt.float32, name="srow0", tag="srow0")
        srow0_v = srow0[:].rearrange(
            "o (u nl k h nh) -> o u nl k h nh", u=BL, nl=4, k=4, h=2,
        )
        DK = [0.0, 1.0, float(W0), float(W0 + 1)]
        for k in range(4):
            for h in range(2):
                nc.vector.tensor_scalar(
                    out=srow0_v[:, :, :, k, h, :],
                    in0=qf0[:].rearrange("o (u nh nl) -> o u nl nh", u=BL, nh=8),
                    scalar1=DK[k] + h * float(HW0), scalar2=None, op0=AL.add,
                )
        widx0 = wrap_idx(srow0, 1024, "L0")
        wkt0 = corner_weights(wx, wy, "L0")
        # wrow0 [1, 1024]: col (u, h, n, k) = wkt0[k, u*32+n]
        wrow0 = pa.tile([1, 1024], dt.float32, name="wrow0", tag="wrow")
        wrow0_v = wrow0[:].rearrange(
            "o (u h n k) -> o u h n k", u=BL, h=2, n=N,
        )
        for k in range(4):
            for h in range(2):
                nc.vector.tensor_copy(
                    out=wrow0_v[:, :, h, :, k],
                    in_=wkt0[:, k * NPTS:(k + 1) * NPTS].rearrange(
                        "o (u n) -> o u n", u=BL,
                    ),
                )
        wb0 = broadcast_weights(wrow0, 1024, "L0", dt.float32)

        # =========== gathers + lerp ===========
        # channel-sum helper: prod [128, 256] -> [1, 128] (fold ch halves)
        def colsum(prod, name):
            ps = ppsum.tile([1, 256], dt.float32, name=f"ps{name}", tag="ps", bufs=2)
            nc.tensor.matmul(ps[:], ones[:], prod[:], start=True, stop=True)
            sb = pool.tile([1, 256], dt.float32, name=f"sb{name}")
            nc.vector.tensor_copy(out=sb[:], in_=ps[:])
            sbv = sb[:].rearrange("o (u h n) -> o u h n", u=BL, h=2)
            r = pool.tile([1, 128], dt.float32, name=f"r{name}")
            rv = r[:].rearrange("o (u n) -> o u n", u=BL)
            nc.vector.tensor_tensor(
                out=rv[:], in0=sbv[:, :, 0, :], in1=sbv[:, :, 1, :], op=AL.add,
            )
            return r

        def level_products(li, done, results):
            """emit ss_li and any newly-available pair dots."""
            prod = pwork.tile([128, 256], dt.float32, name=f"pss{li}", tag="pc")
            nc.vector.tensor_tensor(
                out=prod[:], in0=V[li][:], in1=V[li][:], op=AL.mult,
            )
            results[f"ss{li}"] = colsum(prod, f"ss{li}")
            for (i, j) in PAIRS:
                if li in (i, j) and i in done and j in done:
                    prod = pwork.tile(
                        [128, 256], dt.float32, name=f"pd{i}{j}", tag="pc",
                    )
                    nc.vector.tensor_tensor(
                        out=prod[:], in0=V[i][:], in1=V[j][:], op=AL.mult,
                    )
                    results[f"d{i}{j}"] = colsum(prod, f"d{i}{j}")

        def gather_packed(li):
            H, W = LEVELS[li]
            HW = H * W
            P32, widx, wb = packed[li]
            og = pwork.tile([128, 512], dt.int32, name=f"ogp{li}", tag="ogp")
            nc.gpsimd.ap_gather(
                out_ap=og[:], in_ap=P32[:], idxs_ap=widx[:],
                channels=128, num_elems=8 * HW, d=1, num_idxs=512,
            )
            prod = pwork.tile([128, 1024], dt.float32, name=f"lp{li}", tag="lp")
            nc.vector.tensor_tensor(
                out=prod[:], in0=og[:].bitcast(dt.bfloat16), in1=wb[:], op=AL.mult,
            )
            nc.vector.tensor_reduce(
                out=V[li][:],
                in_=prod[:].rearrange("c (n f) -> c n f", f=4),
                axis=mybir.AxisListType.X, op=AL.add,
            )

        def gather_l0_img(u):
            T = T_tiles[(0, u)]
            og = pwork.tile([128, 256], dt.float32, name=f"og0_{u}", tag="og")
            nc.gpsimd.ap_gather(
                out_ap=og[:], in_ap=T[:], idxs_ap=widx0[:, u * 16:(u + 1) * 16],
                channels=128, num_elems=2 * HW0, d=1, num_idxs=256,
            )
            nc.vector.tensor_tensor(
                out=og[:], in0=og[:], in1=wb0[:, u * 256:(u + 1) * 256],
                op=AL.mult,
            )
            nc.vector.tensor_reduce(
                out=V[0][:, u * 64:(u + 1) * 64],
                in_=og[:].rearrange("c (n f) -> c n f", f=4),
                axis=mybir.AxisListType.X, op=AL.add,
            )

        results = {}
        done = set()
        gather_packed(2)
        done.add(2)
        level_products(2, done, results)
        gather_packed(1)
        done.add(1)
        level_products(1, done, results)
        for u in range(BL):
            gather_l0_img(u)
        done.add(0)
        level_products(0, done, results)

        # =========== cosine epilogue on partition 0 ===========
        rns = []
        for li in range(3):
            nrm = pool.tile([1, 128], dt.float32, name=f"nrm{li}")
            nc.scalar.sqrt(out=nrm[:], in_=results[f"ss{li}"][:])
            nc.vector.tensor_scalar_max(out=nrm[:], in0=nrm[:], scalar1=EPS)
            rn = pool.tile([1, 128], dt.float32, name=f"rn{li}")
            nc.vector.reciprocal(out=rn[:], in_=nrm[:])
            rns.append(rn)

        tot = pool.tile([1, 128], dt.float32)
        first = True
        for i, j in PAIRS:
            t = pool.tile([1, 128], dt.float32, name=f"t{i}{j}")
            nc.vector.tensor_tensor(
                out=t[:], in0=results[f"d{i}{j}"][:], in1=rns[i][:], op=AL.mult,
            )
            nc.vector.tensor_tensor(out=t[:], in0=t[:], in1=rns[j][:], op=AL.mult)
            if first:
                nc.vector.tensor_copy(out=tot[:], in_=t[:])
                first = False
            else:
                nc.vector.tensor_tensor(out=tot[:], in0=tot[:], in1=t[:], op=AL.add)

        res = pool.tile([1, 1], dt.float32)
        nc.vector.tensor_reduce(
            out=res[:], in_=tot[:], axis=mybir.AxisListType.X, op=AL.add
        )
        nc.sync.dma_start(out=out.ap(), in_=res[:])

    nc.compile()
    return nc


def _get_program():
    if "nc" not in _CACHE:
        _CACHE["nc"] = _build_program()
    return _CACHE["nc"]


def _run_device(feat0, feat1, feat2, boxes, **run_kwargs):
    """Shard inputs batch-wise over the 8 cores, run the SPMD program, and
    return the BassKernelResults (one {"out": [1,1]} per core)."""
    from concourse.bass_utils import run_bass_kernel_spmd

    nc = _get_program()

    feats = [
        np.ascontiguousarray(np.asarray(f, dtype=np.float32))
        for f in (feat0, feat1, feat2)
    ]
    boxes = np.ascontiguousarray(np.asarray(boxes, dtype=np.float32))

    in_maps = []
    for k in range(N_CORES):
        sl = slice(k * BL, (k + 1) * BL)
        in_maps.append(
            {
                "feat0": feats[0][sl],
                "feat1": feats[1][sl],
                "feat2": feats[2][sl],
                "boxes": boxes[sl],
            }
        )

    return run_bass_kernel_spmd(
        nc, in_maps, core_ids=list(range(N_CORES)), **run_kwargs
    )


def kernel(feat0, feat1, feat2, boxes):
    r = _run_device(feat0, feat1, feat2, boxes)
    total = np.float64(0.0)
    for m in r.results:
        total += np.float64(m["out"].reshape(-1)[0])

    count = B * N * len(PAIRS)
    avg = np.float32(total) / np.float32(count)
    loss = np.float32(1.0) - avg
    loss = np.nan_to_num(loss, nan=0.0, posinf=1.0, neginf=0.0)
    return np.array(np.clip(loss, 0.0, 2.0), dtype=np.float32)
